# revision 31
# baseline (speedup 1.0000x reference)
"""Trainium2 Bass kernel for chunked flash-attention block (B=2, S=2048, D=1024, H=16).

Sharding: 8 cores = 2 batches x 4 head-groups (4 heads each). Each core computes
its heads' QKV projections + RoPE + per-chunk-softmax attention + its slice of
the output projection; the host sums the 4 partial out-projections per batch.

All device activations are bf16 (fp8 fails the 2e-2 gate: attention output is a
shrinking average, so per-element quantization noise lands full-strength on the
output). The per-head q/k layout puts head_dim on 64-partition blocks
(p = 64*(h%2) + hd) so score matmuls contract over 64 partitions with legal
base partitions {0, 64}.

RoPE pairing is laid out as 32-partition blocks (a-dims in the lower half of
each 64-block, b-dims upper), so the partner swap is four partition-block
copies that run on the otherwise-idle GPSIMD; the cos/sin muls run on DVE with
the sign folded into the per-partition sin table, and GPSIMD does the final
add.

exp() is split between ScalarE (native Exp) and DVE (Schraudolph bit-trick:
i16 = round(x*128/ln2 + 127*128 - C) bitcast to bf16, ~1.8% rms which the
per-chunk softmax ratio mostly tolerates) to keep both engines under the PE
roofline. Units are software-pipelined: scores+exp of unit i+1 are emitted
before W@V+normalize of unit i so the exp engines never starve.
"""

import numpy as np
import ml_dtypes

import concourse.bass as bass
import concourse.tile as tile
from concourse import bacc, mybir
from concourse.bass_utils import run_bass_kernel_spmd
from concourse.masks import make_identity

dt = mybir.dt
F32 = dt.float32
BF16 = dt.bfloat16
I16 = dt.int16
AF = mybir.ActivationFunctionType
OP = mybir.AluOpType

B, S, D, H, HD = 2, 2048, 1024, 16, 64
CHUNK = 1024
NHL = 4              # local heads per core
JL = NHL * HD        # 256 local projected dims
LN2 = float(np.log(2.0))
SC_EXP = HD ** -0.5
C_BIT16 = 7.35
BIT_A = SC_EXP * 128.0 / LN2
BIT_B = 127.0 * 128.0 - C_BIT16

# fraction of exp tiles on ScalarE (rest on DVE bit-exp)
ACT_FRAC = 0.56

_CACHED = {}


def _emit_body(nc, tc, persist, rope, aps, rep):
    (x_d, wq_d, wk_d, wv_d, wo_d, c2_d, s2_d, out_d) = aps
    r = f"r{rep}"

    # ---------------- persistent SBUF tiles + DMA-in --------------------
    x = persist.tile([128, 8, S], BF16, tag="x", name=f"x_{r}")
    x_r = x_d.rearrange("(t p) s -> p t s", p=128)
    nc.sync.dma_start(x[:, :, 0:512], x_r[:, :, 0:512])
    wq = persist.tile([128, 8, 256], BF16, tag="wq", name=f"wq_{r}")
    nc.sync.dma_start(wq[:], wq_d.rearrange("(t p) g -> p t g", p=128))
    wk = persist.tile([128, 8, 256], BF16, tag="wk", name=f"wk_{r}")
    nc.sync.dma_start(wk[:], wk_d.rearrange("(t p) g -> p t g", p=128))
    # cos/sin + late-needed weights go out on the Pool DGE queue so their
    # generation overlaps the SP queue's x/wq/wk stream
    c2h = persist.tile([128, S], BF16, tag="c2h", name=f"c2h_{r}")
    s2h = persist.tile([128, S], BF16, tag="s2h", name=f"s2h_{r}")
    nc.gpsimd.dma_start(c2h[:], c2_d)
    nc.gpsimd.dma_start(s2h[:], s2_d)
    wv = persist.tile([128, 8, 256], BF16, tag="wv", name=f"wv_{r}")
    nc.gpsimd.dma_start(wv[:], wv_d.rearrange("(t p) j -> p t j", p=128))
    wo_sb = persist.tile([128, 2, D], BF16, tag="wo", name=f"wo_{r}")
    nc.gpsimd.dma_start(wo_sb[:], wo_d.rearrange("(t p) n -> p t n", p=128))
    for sb4 in range(1, 4):
        nc.sync.dma_start(x[:, :, sb4 * 512:(sb4 + 1) * 512],
                          x_r[:, :, sb4 * 512:(sb4 + 1) * 512])
    ident = persist.tile([128, 128], BF16, tag="ident", name=f"ident_{r}")
    make_identity(nc, ident[:])

    # rotated q/k, bf16, [128 = 2 heads x 64 hd, S]; hd layout per 64-block:
    # lower 32 partitions = even hd (a), upper 32 = odd hd (b)
    qTrA = persist.tile([128, S], BF16, tag="qTrA", name=f"qTrA_{r}")
    qTrB = persist.tile([128, S], BF16, tag="qTrB", name=f"qTrB_{r}")
    kTrA = persist.tile([128, S], BF16, tag="kTrA", name=f"kTrA_{r}")
    kTrB = persist.tile([128, S], BF16, tag="kTrB", name=f"kTrB_{r}")
    qk_tiles = {("q", 0): qTrA, ("q", 1): qTrB, ("k", 0): kTrA, ("k", 1): kTrB}
    # v + ones-column: [128 sk, chunk 2, sk-tile 8, 4h*65]
    vON = persist.tile([128, 2, 8, 260], BF16, tag="vON", name=f"vON_{r}")
    attn = persist.tile([128, 16, JL], BF16, tag="attn", name=f"attn_{r}")

    vON_on = vON[:].rearrange("p c t (h e) -> p c t h e", e=65)
    nc.gpsimd.memset(vON_on[:, :, :, :, 64:65], 1.0)

    with (
        tc.tile_pool(name=f"sc_{r}", bufs=2, space="PSUM") as scp,
        tc.tile_pool(name=f"psb_{r}", bufs=2, space="PSUM") as psbp,
        tc.tile_pool(name=f"pjx_{r}", bufs=2, space="PSUM") as pjp,
        tc.tile_pool(name=f"et_{r}", bufs=9) as etp,
        tc.tile_pool(name=f"nrm_{r}", bufs=4) as nrmp,
        tc.tile_pool(name=f"osb_{r}", bufs=3) as osbp,
        tc.tile_pool(name=f"at_{r}", bufs=4) as atp,
    ):
        # PE warm-up: HAM clock gate keeps a cold PE at reduced rate for the
        # first ~3us; burn it on the locally-built identity tile so warm-up
        # starts before any DMA lands.
        warm = scp.tile([128, 2, 512], F32, tag="sc", name=f"warm_{r}")
        for i in range(20):
            nc.tensor.matmul(
                warm[:, i % 2, 0:128],
                lhsT=ident[:, 0:128],
                rhs=ident[:, 0:128],
                start=True, stop=True,
            )
        # prefetch ScalarE's Exp table load (~1.3us) behind the DMA window
        twarm = nrmp.tile([128, 2], F32, tag="rec", name=f"twarm_{r}")
        nc.scalar.activation(out=twarm[:, :], in_=ident[:, 0:2], func=AF.Exp)

        proj_ps = {}

        def emit_qk_projmm(which, st, pair):
            """8 projection matmuls for one (q/k, s-tile, head-pair)."""
            wsb = wq if which == "q" else wk
            sl = slice(st * 512, (st + 1) * 512)
            ps = pjp.tile([128, 512], F32, tag="pj", name=f"pj_{r}")
            proj_ps[(which, st, pair)] = ps
            for kt in range(8):
                nc.tensor.matmul(
                    ps[:],
                    lhsT=wsb[:, kt, pair * 128:(pair + 1) * 128],
                    rhs=x[:, kt, sl],
                    start=(kt == 0), stop=(kt == 7),
                )

        rope_cnt = [0]

        def emit_qk_rope(which, st, pair):
            """RoPE for one projected tile: w2 = ps*sin(+-), t2 = ps*cos (DVE);
            u = 32-block swap of w2 (shifted copies); dst = t2 + u. The first
            few tiles gate the whole pipeline, so they get DVE help with the
            swap instead of riding the slower GPSIMD alone."""
            sl = slice(st * 512, (st + 1) * 512)
            ps = proj_ps.pop((which, st, pair))
            idx = rope_cnt[0]
            rope_cnt[0] += 1
            if idx < 2:
                cp_eng = [nc.vector, nc.gpsimd, nc.vector, nc.gpsimd]
                add_eng = nc.vector
            elif idx < 7:
                cp_eng = [nc.vector, nc.gpsimd, nc.gpsimd, nc.gpsimd]
                add_eng = nc.gpsimd
            else:
                cp_eng = [nc.gpsimd] * 4
                add_eng = nc.gpsimd
            w2 = rope.tile([128, 512], BF16, tag="w2", name=f"w2_{r}")
            nc.vector.tensor_mul(w2[:], ps[:], s2h[:, sl])
            t2 = rope.tile([128, 512], BF16, tag="t2", name=f"t2_{r}")
            nc.vector.tensor_mul(t2[:], ps[:], c2h[:, sl])
            u = rope.tile([128, 512], BF16, tag="u", name=f"u_{r}")
            for blk in range(4):
                o = blk * 32
                so = o ^ 32
                cp_eng[blk].tensor_copy(u[o:o + 32, :], w2[so:so + 32, :])
            dst = qk_tiles[(which, pair)]
            add_eng.tensor_add(dst[:, sl], t2[:], u[:])

        def emit_qk_proj(which, st):
            for pair in range(2):
                emit_qk_projmm(which, st, pair)
                emit_qk_rope(which, st, pair)

        def emit_v_tile(st):
            """One [128 sk, 256 j] v-projection tile -> vON."""
            psv = pjp.tile([128, 512], F32, tag="pj", name=f"pv_{r}")
            for kt in range(8):
                nc.tensor.matmul(
                    psv[:, 0:256],
                    lhsT=x[:, kt, st * 128:(st + 1) * 128],
                    rhs=wv[:, kt, :],
                    start=(kt == 0), stop=(kt == 7),
                )
            nc.scalar.copy(
                vON_on[:, st // 8, st % 8, :, 0:64],
                psv[:, 0:256].rearrange("p (h e) -> p h e", e=64),
            )

        exp_cnt = [0, 0]  # [total, on ACT]
        pending = [None]   # previous unit awaiting W@V, drained per-head

        def emit_exp(sc, et, t2i):
            """exp of one [128, 2, 512] score psum tile into et[:, 2t:2t+2]."""
            exp_cnt[0] += 1
            if exp_cnt[1] < ACT_FRAC * exp_cnt[0]:
                exp_cnt[1] += 1
                nc.scalar.activation(out=et[:, 2 * t2i:2 * t2i + 2, :], in_=sc[:],
                                     func=AF.Exp, scale=SC_EXP)
            else:
                nc.vector.tensor_scalar(et[:, 2 * t2i:2 * t2i + 2, :].bitcast(I16),
                                        sc[:], BIT_A, BIT_B, OP.mult, OP.add)

        def emit_scores_exp(c, n, fillers):
            """Scores+exp for all 4 heads at (chunk c, sq block n).
            fillers: list of up to 16 lists of thunks; list h*4+t2i drains
            right after that head's t2i-th score tile, so PE-side filler work
            is spread between score tiles and the exp engines never see a
            long score gap."""
            ets = []
            for h in range(NHL):
                src = qk_tiles[("q", h // 2)]
                ksrc = qk_tiles[("k", h // 2)]
                hb = 64 * (h % 2)
                et = etp.tile([128, 8, 512], BF16, tag="et", name=f"et_{r}")
                ets.append(et)
                for t2i in range(4):
                    sc = scp.tile([128, 2, 512], F32, tag="sc", name=f"sc_{r}")
                    for par in range(2):
                        tg = c * 8 + t2i * 2 + par
                        nc.tensor.matmul(
                            sc[:, par, :],
                            lhsT=ksrc[hb:hb + 64, tg * 128:(tg + 1) * 128],
                            rhs=src[hb:hb + 64, n * 512:(n + 1) * 512],
                            start=True, stop=True,
                        )
                    slot = h * 4 + t2i
                    if slot < len(fillers):
                        for f in fillers[slot]:
                            f()
                    emit_exp(sc, et, t2i)
                if pending[0] is not None:
                    pc, pn, pets = pending[0]
                    emit_wv_norm_m(pc, pn, pets, h)
            return ets

        def emit_out_m(m):
            """Transpose + output projection + store for one sq tile m."""
            at = atp.tile([128, 2, 128], BF16, tag="at", name=f"at_{r}")
            for jt in range(2):
                tp = pjp.tile([128, 128], BF16, tag="pj", name=f"tp_{r}")
                nc.tensor.transpose(tp[:], attn[:, m, jt * 128:(jt + 1) * 128], ident[:])
                nc.vector.tensor_copy(at[:, jt, :], tp[:])
            osb = osbp.tile([128, 2, 512], BF16, tag="osb", name=f"osb_{r}")
            for nn in range(2):
                pso = pjp.tile([128, 512], F32, tag="pj", name=f"po_{r}")
                for jt in range(2):
                    nc.tensor.matmul(
                        pso[:],
                        lhsT=at[:, jt, :],
                        rhs=wo_sb[:, jt, nn * 512:(nn + 1) * 512],
                        start=(jt == 0), stop=(jt == 1),
                    )
                if nn == 0:
                    nc.scalar.copy(osb[:, nn, :], pso[:])
                else:
                    nc.vector.tensor_copy(osb[:, nn, :], pso[:])
            nc.sync.dma_start(
                out_d[m * 128:(m + 1) * 128, :],
                osb[:].rearrange("p a n -> p (a n)"),
            )

        def emit_wv_norm_m(c, n, ets, m2):
            """W@V + per-chunk-softmax normalize + (chunk 1) output projection
            for one m-tile of block n."""
            if True:
                m = n * 4 + m2
                psB = psbp.tile([128, 264], F32, tag="psb", name=f"psB_{r}")
                psBr = psB[:].rearrange("p (h e) -> p h e", e=66)
                for h in range(NHL):
                    for tg in range(8):
                        nc.tensor.matmul(
                            psB[:, h * 66:h * 66 + 65],
                            lhsT=ets[h][:, tg, m2 * 128:(m2 + 1) * 128],
                            rhs=vON[:, c, tg, h * 65:(h + 1) * 65],
                            start=(h == 0 and tg == 0),
                            stop=(h == NHL - 1 and tg == 7),
                            skip_group_check=True,
                        )
                rec = nrmp.tile([128, 4], F32, tag="rec", name=f"rec_{r}")
                nc.vector.reciprocal(rec[:].rearrange("p (h e) -> p h e", e=1),
                                     psBr[:, :, 64:65])
                for h in range(NHL):
                    dstp = attn[:, m, h * 64:(h + 1) * 64]
                    if c == 0:
                        nc.scalar.activation(out=dstp, in_=psBr[:, h, 0:64],
                                             func=AF.Copy, scale=rec[:, h:h + 1])
                    else:
                        nc.vector.scalar_tensor_tensor(
                            out=dstp, in0=psBr[:, h, 0:64], scalar=rec[:, h:h + 1],
                            in1=dstp, op0=OP.mult, op1=OP.add,
                        )
                if c == 1:
                    emit_out_m(m)

        # ---------------- schedule ------------------------------------------
        def pj(which, st, pair):
            return lambda: emit_qk_projmm(which, st, pair)

        def rp(which, st, pair):
            return lambda: emit_qk_rope(which, st, pair)

        def vt(st):
            return lambda: emit_v_tile(st)

        # Only the pair-0 tiles of q0/k0 are rotated before the first unit
        # (heads 0/1); everything else streams through the filler slots one
        # unit ahead of first use.
        emit_qk_projmm("q", 0, 0)
        emit_qk_rope("q", 0, 0)
        emit_qk_projmm("k", 0, 0)
        emit_qk_rope("k", 0, 0)

        # slot layout per unit: [h0s0..h0s3, h1s0.., h2.., h3..]
        # all of vt0-7 must be emitted before the first W@V of unit (0,0),
        # which drains right after head 0 of unit (0,1).
        fill = {
            (0, 0): [[pj("k", 1, 0)], [rp("k", 1, 0)], [pj("q", 0, 1)],
                     [rp("q", 0, 1), pj("k", 0, 1)],
                     [rp("k", 0, 1), pj("k", 1, 1)], [rp("k", 1, 1)],
                     [vt(0)], [vt(1)],
                     [pj("q", 1, 0)], [rp("q", 1, 0)], [pj("q", 1, 1)],
                     [rp("q", 1, 1)], [vt(2)], [vt(3)], [vt(4)], [vt(5)]],
            (0, 1): [[vt(6)], [vt(7)], [pj("q", 2, 0)], [rp("q", 2, 0)],
                     [pj("q", 2, 1)], [rp("q", 2, 1)], [], [], [], [], [], [],
                     [], [], [], []],
            (0, 2): [[pj("q", 3, 0)], [rp("q", 3, 0)], [pj("q", 3, 1)],
                     [rp("q", 3, 1)], [pj("k", 2, 0)], [rp("k", 2, 0)],
                     [pj("k", 2, 1)], [rp("k", 2, 1)], [], [], [], [], [], [], [], []],
            (0, 3): [[pj("k", 3, 0)], [rp("k", 3, 0)], [pj("k", 3, 1)],
                     [rp("k", 3, 1)], [vt(8)], [vt(9)], [vt(10)], [vt(11)],
                     [], [], [], [], [], [], [], []],
            (1, 0): [[vt(12)], [vt(13)], [vt(14)], [vt(15)],
                     [], [], [], [], [], [], [], [], [], [], [], []],
        }
        units = [(c, n) for c in range(2) for n in range(4)]
        for u in units:
            ets = emit_scores_exp(u[0], u[1], fill.get(u, []))
            pending[0] = (u[0], u[1], ets)
        pc, pn, pets = pending[0]
        for m2 in range(4):
            emit_wv_norm_m(pc, pn, pets, m2)


def _build_nc(reps=1):
    nc = bacc.Bacc("TRN2", target_bir_lowering=False, debug=False, num_devices=8)

    aps = (
        nc.dram_tensor("x", [D, S], BF16, kind="ExternalInput").ap(),
        nc.dram_tensor("wq", [D, JL], BF16, kind="ExternalInput").ap(),
        nc.dram_tensor("wk", [D, JL], BF16, kind="ExternalInput").ap(),
        nc.dram_tensor("wv", [D, JL], BF16, kind="ExternalInput").ap(),
        nc.dram_tensor("wo", [JL, D], BF16, kind="ExternalInput").ap(),
        nc.dram_tensor("c2", [128, S], BF16, kind="ExternalInput").ap(),
        nc.dram_tensor("s2", [128, S], BF16, kind="ExternalInput").ap(),
        nc.dram_tensor("out", [S, D], BF16, kind="ExternalOutput").ap(),
    )

    with (
        tile.TileContext(nc) as tc,
        tc.tile_pool(name="persist", bufs=1) as persist,
        tc.tile_pool(name="rope", bufs=3) as rope,
    ):
        for rep in range(reps):
            _emit_body(nc, tc, persist, rope, aps, rep)

    nc.compile()
    return nc


def _get_nc(reps=1):
    if reps not in _CACHED:
        _CACHED[reps] = _build_nc(reps)
    return _CACHED[reps]


def _host_prep(hidden_states, freqs_cis, Wq, Wk, Wv, Wo):
    bf16 = ml_dtypes.bfloat16
    hs = np.asarray(hidden_states, dtype=np.float32)
    fc = np.asarray(freqs_cis, dtype=np.float32)
    Wq = np.asarray(Wq, dtype=np.float32)
    Wk = np.asarray(Wk, dtype=np.float32)
    Wv = np.asarray(Wv, dtype=np.float32)
    Wo = np.asarray(Wo, dtype=np.float32)

    # per-partition cos/sin for hd layout p = 64*hpair + 32*(odd) + f:
    # lower 32 of each 64-block = even hd (freq f = p%32), upper 32 = odd hd.
    # sign: +sin on the a-block (its partner u comes from the b-block and
    # carries -sin), see kernel docstring.
    cos, sin = fc[:, :, 0], fc[:, :, 1]               # [S, 32]
    f_idx = np.arange(128) % 32
    sign = np.where((np.arange(128) % 64) < 32, 1.0, -1.0).astype(np.float32)
    c2 = np.ascontiguousarray(cos.T[f_idx]).astype(bf16)            # [128, S]
    s2 = np.ascontiguousarray(sin.T[f_idx] * sign[:, None]).astype(bf16)

    xTs = [np.ascontiguousarray(hs[b].T).astype(bf16) for b in range(B)]

    in_maps = []
    for core in range(8):
        b, g = core // 4, core % 4
        jbase = g * JL
        # q/k col perm: per head, evens then odds (a-block, b-block)
        perm = []
        for h in range(NHL):
            perm += [jbase + h * 64 + 2 * f for f in range(32)]
            perm += [jbase + h * 64 + 2 * f + 1 for f in range(32)]
        perm = np.array(perm)
        in_maps.append({
            "x": xTs[b],
            "wq": np.ascontiguousarray(Wq[:, perm]).astype(bf16),
            "wk": np.ascontiguousarray(Wk[:, perm]).astype(bf16),
            "wv": np.ascontiguousarray(Wv[:, jbase:jbase + JL]).astype(bf16),
            "wo": np.ascontiguousarray(Wo[jbase:jbase + JL, :]).astype(bf16),
            "c2": c2,
            "s2": s2,
        })
    return in_maps


def kernel(hidden_states, freqs_cis, Wq, Wk, Wv, Wo, _trace=False, _reps=1):
    nc = _get_nc(_reps)
    in_maps = _host_prep(hidden_states, freqs_cis, Wq, Wk, Wv, Wo)
    if _trace:
        try:
            from antenv.axon_hooks import get_axon_ntff_profile_hook  # noqa: F401
        except ImportError:
            _trace = False
    res = run_bass_kernel_spmd(nc, in_maps, core_ids=list(range(8)), trace=_trace)
    outs = [r["out"].astype(np.float32) for r in res.results]
    full = np.zeros((B, S, D), dtype=np.float32)
    for core in range(8):
        full[core // 4] += outs[core]
    if _trace:
        kernel._last_results = res
    return full


# revision 62
# speedup vs baseline: 1.0399x; 1.0399x over previous
"""Trainium2 Bass kernel for chunked flash-attention block (B=2, S=2048, D=1024, H=16).

Sharding: 8 cores = 2 batches x 4 head-groups (4 heads each). Each core computes
its heads' QKV projections + RoPE + per-chunk-softmax attention + its slice of
the output projection; the host sums the 4 partial out-projections per batch.

All device activations are bf16 (fp8 fails the 2e-2 gate: attention output is a
shrinking average, so per-element quantization noise lands full-strength on the
output). The per-head q/k layout puts head_dim on 64-partition blocks
(p = 64*(h%2) + hd) so score matmuls contract over 64 partitions with legal
base partitions {0, 64}.

RoPE pairing is laid out as 32-partition blocks (a-dims in the lower half of
each 64-block, b-dims upper), so the partner swap is four partition-block
copies that run on the otherwise-idle GPSIMD; the cos/sin muls run on DVE with
the sign folded into the per-partition sin table, and GPSIMD does the final
add.

exp() is split between ScalarE (native Exp) and DVE (Schraudolph bit-trick:
i16 = round(x*128/ln2 + 127*128 - C) bitcast to bf16, ~1.8% rms which the
per-chunk softmax ratio mostly tolerates) to keep both engines under the PE
roofline. Units are software-pipelined: scores+exp of unit i+1 are emitted
before W@V+normalize of unit i so the exp engines never starve.
"""

import numpy as np
import ml_dtypes

import concourse.bass as bass
import concourse.tile as tile
from concourse import bacc, mybir
from concourse.bass_utils import run_bass_kernel_spmd
from concourse.masks import make_identity

dt = mybir.dt
F32 = dt.float32
BF16 = dt.bfloat16
I16 = dt.int16
AF = mybir.ActivationFunctionType
OP = mybir.AluOpType

B, S, D, H, HD = 2, 2048, 1024, 16, 64
CHUNK = 1024
NHL = 4              # local heads per core
JL = NHL * HD        # 256 local projected dims
LN2 = float(np.log(2.0))
SC_EXP = HD ** -0.5
C_BIT16 = 7.35
BIT_A = SC_EXP * 128.0 / LN2
BIT_B = 127.0 * 128.0 - C_BIT16

# fraction of exp tiles on ScalarE (rest on DVE bit-exp)
ACT_FRAC = 0.56

_CACHED = {}


def _emit_body(nc, tc, persist, rope, aps, rep):
    (x_d, wq_d, wk_d, wv_d, wo_d, c2_d, s2_d, id_d, out_d) = aps
    r = f"r{rep}"

    # ---------------- persistent SBUF tiles + DMA-in --------------------
    x = persist.tile([128, S, 8], BF16, tag="x", name=f"x_{r}")
    x_r = x_d.rearrange("p (s t) -> p s t", t=8)
    wq = persist.tile([128, 8, 256], BF16, tag="wq", name=f"wq_{r}")
    nc.sync.dma_start(wq[:], wq_d.rearrange("p (t g) -> p t g", g=256))
    wk = persist.tile([128, 8, 256], BF16, tag="wk", name=f"wk_{r}")
    nc.sync.dma_start(wk[:], wk_d.rearrange("p (t g) -> p t g", g=256))
    # identity (feeds the PE warm-up + transposes) arrives as the first tiny
    # SP-queue DMA so warm-up starts immediately and GPSIMD stays free
    ident = persist.tile([128, 128], BF16, tag="ident", name=f"ident_{r}")
    nc.sync.dma_start(ident[:], id_d)
    nc.sync.dma_start(x[:, 0:512, :], x_r[:, 0:512, :])
    # cos/sin + late-needed weights go out on the Pool DGE queue so their
    # generation overlaps the SP queue's x/wq/wk stream
    c2h = persist.tile([128, S], BF16, tag="c2h", name=f"c2h_{r}")
    s2h = persist.tile([128, S], BF16, tag="s2h", name=f"s2h_{r}")
    nc.gpsimd.dma_start(c2h[:], c2_d)
    nc.gpsimd.dma_start(s2h[:], s2_d)
    wv = persist.tile([128, 8, 256], BF16, tag="wv", name=f"wv_{r}")
    nc.gpsimd.dma_start(wv[:], wv_d.rearrange("p (t j) -> p t j", j=256))
    wo_sb = persist.tile([128, 2, D], BF16, tag="wo", name=f"wo_{r}")
    nc.gpsimd.dma_start(wo_sb[:], wo_d.rearrange("(t p) n -> p t n", p=128))
    for sb4 in range(1, 4):
        nc.sync.dma_start(x[:, sb4 * 512:(sb4 + 1) * 512, :],
                          x_r[:, sb4 * 512:(sb4 + 1) * 512, :])

    # rotated q/k, bf16, [128 = 2 heads x 64 hd, S]; hd layout per 64-block:
    # lower 32 partitions = even hd (a), upper 32 = odd hd (b)
    qTrA = persist.tile([128, S], BF16, tag="qTrA", name=f"qTrA_{r}")
    qTrB = persist.tile([128, S], BF16, tag="qTrB", name=f"qTrB_{r}")
    kTrA = persist.tile([128, S], BF16, tag="kTrA", name=f"kTrA_{r}")
    kTrB = persist.tile([128, S], BF16, tag="kTrB", name=f"kTrB_{r}")
    qk_tiles = {("q", 0): qTrA, ("q", 1): qTrB, ("k", 0): kTrA, ("k", 1): kTrB}
    # v + ones-column: [128 sk, chunk 2, sk-tile 8, 4h*65]
    vON = persist.tile([128, 2, 8, 260], BF16, tag="vON", name=f"vON_{r}")
    attn = persist.tile([128, 16, JL], BF16, tag="attn", name=f"attn_{r}")

    vON_on = vON[:].rearrange("p c t (h e) -> p c t h e", e=65)
    nc.gpsimd.memset(vON_on[:, :, :, :, 64:65], 1.0)

    with (
        tc.tile_pool(name=f"sc_{r}", bufs=2, space="PSUM") as scp,
        tc.tile_pool(name=f"psb_{r}", bufs=2, space="PSUM") as psbp,
        tc.tile_pool(name=f"pjx_{r}", bufs=2, space="PSUM") as pjp,
        tc.tile_pool(name=f"et_{r}", bufs=9) as etp,
        tc.tile_pool(name=f"nrm_{r}", bufs=4) as nrmp,
    ):
        osbp = nrmp
        atp = nrmp
        # PE warm-up: HAM clock gate keeps a cold PE at reduced rate for the
        # first ~3us; burn it on the locally-built identity tile so warm-up
        # starts before any DMA lands.
        warm = scp.tile([128, 2, 512], F32, tag="sc", name=f"warm_{r}")
        for i in range(20):
            nc.tensor.matmul(
                warm[:, i % 2, 0:128],
                lhsT=ident[:, 0:128],
                rhs=ident[:, 0:128],
                start=True, stop=True,
            )
        # prefetch ScalarE's Exp table load (~1.3us) behind the DMA window
        twarm = nrmp.tile([128, 2], F32, tag="rec", name=f"twarm_{r}")
        nc.scalar.activation(out=twarm[:, :], in_=ident[:, 0:2], func=AF.Exp)

        proj_ps = {}

        def emit_qk_projmm(which, st, pair):
            """8 projection matmuls for one (q/k, s-tile, head-pair)."""
            wsb = wq if which == "q" else wk
            sl = slice(st * 512, (st + 1) * 512)
            ps = pjp.tile([128, 512], F32, tag="pj", name=f"pj_{r}")
            proj_ps[(which, st, pair)] = ps
            for kt in range(8):
                nc.tensor.matmul(
                    ps[:],
                    lhsT=wsb[:, kt, pair * 128:(pair + 1) * 128],
                    rhs=x[:, sl, kt],
                    start=(kt == 0), stop=(kt == 7),
                )

        rope_cnt = [0]

        def emit_qk_rope(which, st, pair):
            """RoPE for one projected tile: w2 = ps*sin(+-), t2 = ps*cos (DVE);
            u = 32-block swap of w2 (shifted copies); dst = t2 + u. The first
            few tiles gate the whole pipeline, so they get DVE help with the
            swap instead of riding the slower GPSIMD alone."""
            sl = slice(st * 512, (st + 1) * 512)
            ps = proj_ps.pop((which, st, pair))
            idx = rope_cnt[0]
            rope_cnt[0] += 1
            if idx < 2:
                cp_eng = [nc.vector, nc.gpsimd, nc.vector, nc.gpsimd]
                add_eng = nc.vector
            elif idx < 7:
                cp_eng = [nc.vector, nc.gpsimd, nc.gpsimd, nc.gpsimd]
                add_eng = nc.gpsimd
            else:
                cp_eng = [nc.gpsimd] * 4
                add_eng = nc.gpsimd
            w2 = rope.tile([128, 512], BF16, tag="w2", name=f"w2_{r}")
            nc.vector.tensor_mul(w2[:], ps[:], s2h[:, sl])
            t2 = rope.tile([128, 512], BF16, tag="t2", name=f"t2_{r}")
            nc.vector.tensor_mul(t2[:], ps[:], c2h[:, sl])
            u = rope.tile([128, 512], BF16, tag="u", name=f"u_{r}")
            for blk in range(4):
                o = blk * 32
                so = o ^ 32
                cp_eng[blk].tensor_copy(u[o:o + 32, :], w2[so:so + 32, :])
            dst = qk_tiles[(which, pair)]
            add_eng.tensor_add(dst[:, sl], t2[:], u[:])

        def emit_qk_proj(which, st):
            for pair in range(2):
                emit_qk_projmm(which, st, pair)
                emit_qk_rope(which, st, pair)

        def emit_v_tile(st):
            """One [128 sk, 256 j] v-projection tile -> vON."""
            psv = pjp.tile([128, 512], F32, tag="pj", name=f"pv_{r}")
            for kt in range(8):
                nc.tensor.matmul(
                    psv[:, 0:256],
                    lhsT=x[:, st * 128:(st + 1) * 128, kt],
                    rhs=wv[:, kt, :],
                    start=(kt == 0), stop=(kt == 7),
                )
            nc.scalar.copy(
                vON_on[:, st // 8, st % 8, :, 0:64],
                psv[:, 0:256].rearrange("p (h e) -> p h e", e=64),
            )

        exp_cnt = [0, 0]  # [total, on ACT]
        pending = [None]   # previous unit awaiting W@V, drained per-head

        def emit_exp(sc, et, t2i):
            """exp of one [128, 2, 512] score psum tile into et[:, 2t:2t+2]."""
            exp_cnt[0] += 1
            if exp_cnt[1] < ACT_FRAC * exp_cnt[0]:
                exp_cnt[1] += 1
                nc.scalar.activation(out=et[:, 2 * t2i:2 * t2i + 2, :], in_=sc[:],
                                     func=AF.Exp, scale=SC_EXP)
            else:
                nc.vector.tensor_scalar(et[:, 2 * t2i:2 * t2i + 2, :].bitcast(I16),
                                        sc[:], BIT_A, BIT_B, OP.mult, OP.add)

        def emit_scores_exp(c, n, fillers):
            """Scores+exp for all 4 heads at (chunk c, sq block n).
            fillers: list of up to 16 lists of thunks; list h*4+t2i drains
            right after that head's t2i-th score tile, so PE-side filler work
            is spread between score tiles and the exp engines never see a
            long score gap."""
            ets = []
            for h in range(NHL):
                src = qk_tiles[("q", h // 2)]
                ksrc = qk_tiles[("k", h // 2)]
                hb = 64 * (h % 2)
                et = etp.tile([128, 8, 512], BF16, tag="et", name=f"et_{r}")
                ets.append(et)
                for t2i in range(4):
                    sc = scp.tile([128, 2, 512], F32, tag="sc", name=f"sc_{r}")
                    for par in range(2):
                        tg = c * 8 + t2i * 2 + par
                        nc.tensor.matmul(
                            sc[:, par, :],
                            lhsT=ksrc[hb:hb + 64, tg * 128:(tg + 1) * 128],
                            rhs=src[hb:hb + 64, n * 512:(n + 1) * 512],
                            start=True, stop=True,
                        )
                    slot = h * 4 + t2i
                    if slot < len(fillers):
                        for f in fillers[slot]:
                            f()
                    emit_exp(sc, et, t2i)
                if pending[0] is not None:
                    pc, pn, pets = pending[0]
                    emit_wv_norm_m(pc, pn, pets, h)
            return ets

        def emit_out_m(m):
            """Transpose + output projection + store for one sq tile m."""
            at = atp.tile([128, 2, 128], BF16, tag="at", name=f"at_{r}")
            for jt in range(2):
                tp = pjp.tile([128, 128], BF16, tag="pj", name=f"tp_{r}")
                nc.tensor.transpose(tp[:], attn[:, m, jt * 128:(jt + 1) * 128], ident[:])
                nc.vector.tensor_copy(at[:, jt, :], tp[:])
            osb = osbp.tile([128, 2, 512], BF16, tag="osb", name=f"osb_{r}")
            for nn in range(2):
                pso = pjp.tile([128, 512], F32, tag="pj", name=f"po_{r}")
                for jt in range(2):
                    nc.tensor.matmul(
                        pso[:],
                        lhsT=at[:, jt, :],
                        rhs=wo_sb[:, jt, nn * 512:(nn + 1) * 512],
                        start=(jt == 0), stop=(jt == 1),
                    )
                if nn == 0:
                    nc.scalar.copy(osb[:, nn, :], pso[:])
                else:
                    nc.vector.tensor_copy(osb[:, nn, :], pso[:])
            nc.sync.dma_start(
                out_d[m * 128:(m + 1) * 128, :],
                osb[:].rearrange("p a n -> p (a n)"),
            )

        def emit_wv_norm_m(c, n, ets, m2):
            """W@V + per-chunk-softmax normalize + (chunk 1) output projection
            for one m-tile of block n."""
            if True:
                m = n * 4 + m2
                psB = psbp.tile([128, 264], F32, tag="psb", name=f"psB_{r}")
                psBr = psB[:].rearrange("p (h e) -> p h e", e=66)
                for h in range(NHL):
                    for tg in range(8):
                        nc.tensor.matmul(
                            psB[:, h * 66:h * 66 + 65],
                            lhsT=ets[h][:, tg, m2 * 128:(m2 + 1) * 128],
                            rhs=vON[:, c, tg, h * 65:(h + 1) * 65],
                            start=(h == 0 and tg == 0),
                            stop=(h == NHL - 1 and tg == 7),
                            skip_group_check=True,
                        )
                # copy the raw accumulators to SBUF immediately (frees the
                # psB bank after one op) and normalize from there on GPSIMD,
                # which keeps the exp engines' queues clear of norm work
                braw = nrmp.tile([128, 264], F32, tag="braw", name=f"braw_{r}")
                if m % 2 == 0:
                    nc.scalar.copy(braw[:], psB[:])
                else:
                    nc.vector.tensor_copy(braw[:], psB[:])
                brr = braw[:].rearrange("p (h e) -> p h e", e=66)
                rec = nrmp.tile([128, 4], F32, tag="rec", name=f"rec_{r}")
                nc.vector.reciprocal(rec[:].rearrange("p (h e) -> p h e", e=1),
                                     brr[:, :, 64:65])
                for h in range(NHL):
                    dstp = attn[:, m, h * 64:(h + 1) * 64]
                    if c == 0:
                        nc.scalar.activation(out=dstp, in_=brr[:, h, 0:64],
                                             func=AF.Copy, scale=rec[:, h:h + 1])
                    else:
                        nc.vector.scalar_tensor_tensor(
                            out=dstp, in0=brr[:, h, 0:64], scalar=rec[:, h:h + 1],
                            in1=dstp, op0=OP.mult, op1=OP.add,
                        )
                if c == 1:
                    emit_out_m(m)

        # ---------------- schedule ------------------------------------------
        def pj(which, st, pair):
            return lambda: emit_qk_projmm(which, st, pair)

        def rp(which, st, pair):
            return lambda: emit_qk_rope(which, st, pair)

        def vt(st):
            return lambda: emit_v_tile(st)

        # Only the pair-0 tiles of q0/k0 are rotated before the first unit
        # (heads 0/1); everything else streams through the filler slots one
        # unit ahead of first use.
        emit_qk_projmm("q", 0, 0)
        emit_qk_rope("q", 0, 0)
        emit_qk_projmm("k", 0, 0)
        emit_qk_rope("k", 0, 0)

        # slot layout per unit: [h0s0..h0s3, h1s0.., h2.., h3..]
        # all of vt0-7 must be emitted before the first W@V of unit (0,0),
        # which drains right after head 0 of unit (0,1).
        fill = {
            (0, 0): [[pj("k", 1, 0)], [rp("k", 1, 0)], [pj("q", 0, 1)],
                     [rp("q", 0, 1), pj("k", 0, 1)],
                     [rp("k", 0, 1), pj("k", 1, 1)], [rp("k", 1, 1)],
                     [vt(0)], [vt(1)],
                     [pj("q", 1, 0)], [rp("q", 1, 0)], [pj("q", 1, 1)],
                     [rp("q", 1, 1)], [vt(2)], [vt(3)], [vt(4)], [vt(5)]],
            (0, 1): [[vt(6)], [vt(7)], [pj("q", 2, 0)], [rp("q", 2, 0)],
                     [pj("q", 2, 1)], [rp("q", 2, 1)], [], [], [], [], [], [],
                     [], [], [], []],
            (0, 2): [[pj("q", 3, 0)], [rp("q", 3, 0)], [pj("q", 3, 1)],
                     [rp("q", 3, 1)], [pj("k", 2, 0)], [rp("k", 2, 0)],
                     [pj("k", 2, 1)], [rp("k", 2, 1)], [], [], [], [], [], [], [], []],
            (0, 3): [[pj("k", 3, 0)], [rp("k", 3, 0)], [pj("k", 3, 1)],
                     [rp("k", 3, 1)], [vt(8)], [vt(9)], [vt(10)], [vt(11)],
                     [], [], [], [], [], [], [], []],
            (1, 0): [[vt(12)], [vt(13)], [vt(14)], [vt(15)],
                     [], [], [], [], [], [], [], [], [], [], [], []],
        }
        units = [(c, n) for c in range(2) for n in range(4)]
        for u in units:
            ets = emit_scores_exp(u[0], u[1], fill.get(u, []))
            pending[0] = (u[0], u[1], ets)
        pc, pn, pets = pending[0]
        for m2 in range(4):
            emit_wv_norm_m(pc, pn, pets, m2)


def _build_nc(reps=1):
    nc = bacc.Bacc("TRN2", target_bir_lowering=False, debug=False, num_devices=8)

    aps = (
        nc.dram_tensor("x", [128, S * 8], BF16, kind="ExternalInput").ap(),
        nc.dram_tensor("wq", [128, 8 * JL], BF16, kind="ExternalInput").ap(),
        nc.dram_tensor("wk", [128, 8 * JL], BF16, kind="ExternalInput").ap(),
        nc.dram_tensor("wv", [128, 8 * JL], BF16, kind="ExternalInput").ap(),
        nc.dram_tensor("wo", [JL, D], BF16, kind="ExternalInput").ap(),
        nc.dram_tensor("c2", [128, S], BF16, kind="ExternalInput").ap(),
        nc.dram_tensor("s2", [128, S], BF16, kind="ExternalInput").ap(),
        nc.dram_tensor("ident", [128, 128], BF16, kind="ExternalInput").ap(),
        nc.dram_tensor("out", [S, D], BF16, kind="ExternalOutput").ap(),
    )

    with (
        tile.TileContext(nc) as tc,
        tc.tile_pool(name="persist", bufs=1) as persist,
        tc.tile_pool(name="rope", bufs=4) as rope,
    ):
        for rep in range(reps):
            _emit_body(nc, tc, persist, rope, aps, rep)

    nc.compile()
    return nc


def _get_nc(reps=1):
    if reps not in _CACHED:
        _CACHED[reps] = _build_nc(reps)
    return _CACHED[reps]


def _host_prep(hidden_states, freqs_cis, Wq, Wk, Wv, Wo):
    bf16 = ml_dtypes.bfloat16
    hs = np.asarray(hidden_states, dtype=np.float32)
    fc = np.asarray(freqs_cis, dtype=np.float32)
    Wq = np.asarray(Wq, dtype=np.float32)
    Wk = np.asarray(Wk, dtype=np.float32)
    Wv = np.asarray(Wv, dtype=np.float32)
    Wo = np.asarray(Wo, dtype=np.float32)

    # per-partition cos/sin for hd layout p = 64*hpair + 32*(odd) + f:
    # lower 32 of each 64-block = even hd (freq f = p%32), upper 32 = odd hd.
    # sign: +sin on the a-block (its partner u comes from the b-block and
    # carries -sin), see kernel docstring.
    cos, sin = fc[:, :, 0], fc[:, :, 1]               # [S, 32]
    f_idx = np.arange(128) % 32
    sign = np.where((np.arange(128) % 64) < 32, 1.0, -1.0).astype(np.float32)
    c2 = np.ascontiguousarray(cos.T[f_idx]).astype(bf16)            # [128, S]
    s2 = np.ascontiguousarray(sin.T[f_idx] * sign[:, None]).astype(bf16)

    # dram layouts are per-partition contiguous: x[p, s, t], w[p, t, g]
    xTs = [np.ascontiguousarray(
        hs[b].T.reshape(8, 128, S).transpose(1, 2, 0).reshape(128, S * 8)
    ).astype(bf16) for b in range(B)]

    def packw(w):
        return np.ascontiguousarray(
            w.reshape(8, 128, JL).transpose(1, 0, 2).reshape(128, 8 * JL)
        ).astype(bf16)

    in_maps = []
    for core in range(8):
        b, g = core // 4, core % 4
        jbase = g * JL
        # q/k col perm: per head, evens then odds (a-block, b-block)
        perm = []
        for h in range(NHL):
            perm += [jbase + h * 64 + 2 * f for f in range(32)]
            perm += [jbase + h * 64 + 2 * f + 1 for f in range(32)]
        perm = np.array(perm)
        in_maps.append({
            "x": xTs[b],
            "wq": packw(Wq[:, perm]),
            "wk": packw(Wk[:, perm]),
            "wv": packw(Wv[:, jbase:jbase + JL]),
            "wo": np.ascontiguousarray(Wo[jbase:jbase + JL, :]).astype(bf16),
            "c2": c2,
            "s2": s2,
            "ident": np.eye(128, dtype=np.float32).astype(bf16),
        })
    return in_maps


def kernel(hidden_states, freqs_cis, Wq, Wk, Wv, Wo, _trace=False, _reps=1):
    nc = _get_nc(_reps)
    in_maps = _host_prep(hidden_states, freqs_cis, Wq, Wk, Wv, Wo)
    if _trace:
        try:
            from antenv.axon_hooks import get_axon_ntff_profile_hook  # noqa: F401
        except ImportError:
            _trace = False
    res = run_bass_kernel_spmd(nc, in_maps, core_ids=list(range(8)), trace=_trace)
    outs = [r["out"].astype(np.float32) for r in res.results]
    full = np.zeros((B, S, D), dtype=np.float32)
    for core in range(8):
        full[core // 4] += outs[core]
    if _trace:
        kernel._last_results = res
    return full
